# revision 3
# baseline (speedup 1.0000x reference)
"""Trainium2 Bass kernel (v5: x cast via DRAM->DRAM SWDGE cast-DMA, scalar ring = y only) for BitFlipLinear: y[b,s,o] = sum_i x[b,s,i]*W[o,i] + bias[o].

Data-parallel over batch: each of the 8 NeuronCores computes one
[4096,4096] @ [4096,4096]^T matmul (137 GFLOP/core).

v3 (over v2's single-bf16-pass):
  - W^T fp8 residency is built WITHOUT the DRAM bf16 scratch: SWDGE
    cast-DMA loads W f32->bf16 straight into SBUF (o on partitions),
    PE transpose-mode flips each 128x128 block into PSUM, DVE evicts
    with an f32->fp8 cast into the resident W^T.  W HBM traffic drops
    128MB -> 64MB and the serial ~210us cast prologue disappears; the
    sync HWDGE ring now carries ONLY the per-s-tile x transposes.
  - bias: single-row bf16 + K=1 replication matmuls (no SBUF->SBUF
    DMAs, which Tile serializes against transpose-DMAs).
  - PSUM: 3 product slots x 2 banks + 2 transpose banks = 8.

Numerics: W in {0,1,3} is exact in bf16/fp8; x is one bf16 pass
(rel err ~1.7e-3 << 2e-2 gate); accumulation in f32 PSUM.
"""

import os
import sys

for _p in ("/opt/trn_rl_repo",):
    if os.path.isdir(_p) and _p not in sys.path:
        sys.path.append(_p)

import numpy as np

B, S, K, O = 8, 4096, 4096, 4096
N_CORES = 8
CONV_I = 256
BIAS_CH = 256

_NC_CACHE = {}


def build_nc(S=S, K=K, O=O, enable_asserts=False, repeat=1):
    import concourse.bacc as bacc
    import concourse.tile as tile
    import concourse.mybir as mybir

    f32 = mybir.dt.float32
    bf16 = mybir.dt.bfloat16
    fp8 = mybir.dt.float8e4
    P = 128
    ST = S // P
    KO = K // P
    NB = O // 512             # 8 o-banks of 512
    NQ = 4                    # product quarters per s-tile (2 banks each)

    nc = bacc.Bacc("TRN2", target_bir_lowering=False, debug=False,
                   enable_asserts=enable_asserts)

    ap_x = nc.dram_tensor("x", [S, K], f32, kind="ExternalInput").ap()
    ap_w = nc.dram_tensor("w", [O, K], f32, kind="ExternalInput").ap()
    ap_bias = nc.dram_tensor("bias", [O], f32, kind="ExternalInput").ap()
    ap_id = nc.dram_tensor("ident", [P, P], bf16, kind="ExternalInput").ap()
    ap_y = nc.dram_tensor("y", [S, O], f32, kind="ExternalOutput").ap()

    with tile.TileContext(nc) as tc:
        with (
            tc.tile_pool(name="dram", bufs=1, space="DRAM") as dram,
            tc.tile_pool(name="const", bufs=1) as const,
            tc.tile_pool(name="bstage", bufs=2) as bstage,
            tc.tile_pool(name="wres", bufs=1) as wresp,
            tc.tile_pool(name="wcp", bufs=7) as wcp,
            tc.tile_pool(name="xts", bufs=3) as xtsp,
            tc.tile_pool(name="outp", bufs=2) as outp,
            tc.tile_pool(name="psum", bufs=3, space="PSUM") as psum,
            tc.tile_pool(name="psumT", bufs=2, space="PSUM") as psumT,
        ):
            xh = dram.tile([S, K], bf16)

            for _rep in range(repeat):
              ident = const.tile([P, P], bf16)
              nc.scalar.dma_start(ident[:], ap_id[:, :])

              ones1 = const.tile([1, P], bf16)
              nc.vector.memset(ones1[:], 1.0)

              # W^T fp8 residency: per o-bank (512 rows of W):
              #   8 cast-DMA chunks [128 o, 2048 k] f32->bf16 into SBUF,
              #   PE-transpose 128x128 blocks -> PSUM [128k, 512o],
              #   DVE copy (f32->fp8) -> wres8[:, ko, bank]
              wres8 = wresp.tile([P, KO, O], fp8)
              KH = 2048
              for b in range(NB):
                wcs = {}
                for i in range(4):
                  for h in range(2):
                    wc = wcp.tile([P, KH], bf16, tag="wc")
                    r0 = (4 * b + i) * P
                    nc.gpsimd.dma_start(
                        wc[:], ap_w[r0:r0 + P, h * KH:(h + 1) * KH])
                    wcs[(i, h)] = wc[:].rearrange("p (kl q) -> p kl q", q=P)
                for ko in range(KO):
                  h, kl = divmod(ko, KH // P)
                  tp = psumT.tile([P, 512], bf16, tag="tp")
                  for i in range(4):
                    nc.tensor.transpose(
                        tp[:, i * P:(i + 1) * P], wcs[(i, h)][:, kl, :], ident[:])
                  nc.vector.tensor_copy(
                      wres8[:, ko, b * 512:(b + 1) * 512], tp[:])

              # bias_rep[p, o] = bf16(bias)[o], replicated via K=1 matmuls
              bias_rep = const.tile([P, O], mybir.dt.float16)
              for g in range(NB):
                  sl = slice(g * 512, (g + 1) * 512)
                  bst = bstage.tile([1, 512], f32)
                  nc.scalar.dma_start(bst[:], ap_bias[None, sl])
                  b1 = bstage.tile([1, 512], bf16, tag="b1")
                  nc.vector.tensor_copy(b1[:], bst[:])
                  bp = psumT.tile([P, 512], f32, tag="tp")
                  nc.tensor.matmul(bp[:], ones1[:], b1[:], start=True, stop=True)
                  nc.vector.tensor_copy(bias_rep[:, sl], bp[:])

              # s-tiles
              for st in range(ST):
                rows = slice(st * P, (st + 1) * P)
                # x f32 -> bf16 straight in DRAM (one SWDGE cast-DMA)
                nc.gpsimd.dma_start(xh[rows, :], ap_x[rows, :])

                xt = xtsp.tile([P, KO, P], bf16)
                nc.sync.dma_start(xt[:], xh[rows, :], transpose=True)

                for q in range(NQ):
                    pt = psum.tile([P, 1024], f32)
                    for obl in range(2):
                        b0 = q * 1024 + obl * 512
                        bank = pt[:, obl * 512:(obl + 1) * 512]
                        for ko in range(KO):
                            nc.tensor.matmul(
                                bank,
                                xt[:, ko, :],
                                wres8[:, ko, b0:b0 + 512],
                                start=(ko == 0), stop=(ko == KO - 1),
                            )
                    ot = outp.tile([P, 1024], f32)
                    o0 = q * 1024
                    nc.any.tensor_add(
                        ot[:], pt[:], bias_rep[:, o0:o0 + 1024])
                    nc.scalar.dma_start(ap_y[rows, o0:o0 + 1024], ot[:])

    nc.compile()
    return nc


def _get_nc():
    key = (S, K, O)
    if key not in _NC_CACHE:
        _NC_CACHE[key] = build_nc(S, K, O)
    return _NC_CACHE[key]


def _identity():
    import ml_dtypes
    return np.eye(128, dtype=ml_dtypes.bfloat16)


def make_in_maps(x, weight, bias):
    x = np.ascontiguousarray(np.asarray(x, dtype=np.float32))
    weight = np.ascontiguousarray(np.asarray(weight, dtype=np.float32))
    bias = np.ascontiguousarray(np.asarray(bias, dtype=np.float32))
    assert x.shape == (B, S, K), x.shape
    ident = _identity()
    return [
        {"x": np.ascontiguousarray(x[b]), "w": weight, "bias": bias,
         "ident": ident}
        for b in range(B)
    ]


def kernel(x, weight, bias):
    from concourse.bass_utils import run_bass_kernel_spmd

    nc = _get_nc()
    in_maps = make_in_maps(x, weight, bias)
    res = run_bass_kernel_spmd(nc, in_maps, core_ids=list(range(N_CORES)))
    return np.stack([res.results[b]["y"] for b in range(B)], axis=0).astype(np.float32)
